# revision 30
# baseline (speedup 1.0000x reference)
"""AxialAttention Trainium2 kernel: 8-core SPMD, no collectives.

Sharding: core (b, j) computes height-attention for x[b, :, 64j:64j+64, :]
and width-attention for x[b, 32j:32j+32, :, :]; host sums partial outputs.

Per-phase on-device dataflow (all matmuls bf16, fp32 PSUM):
  xT resident [C=256 (2 part-chunks), tokens=8192]
  qT,kT = W.T @ x   (lhsT = W chunks)            [256, 8192] bf16
  v     = x @ Wv    (lhsT = xT token-tiles), N=256 -> strided copy into
          zeroed v_aug tile [128, 8*64] (per-head [v_h | 1 | 0...])
  scores sT = k-stationary, 4-head row-group packing (K=32)
  aT = exp(scale * sT)  on ScalarE, PSUM->SBUF bf16
  AV: stationary [v_h | 1 | 0] (M=64), 2-head col-group packing -> [ohT_h; denom_h]
  denom rows DMA-compacted (one strided DMA) -> one DVE reciprocal per half-phase
  recip rows DMA partition-broadcast (one DMA) -> bc tiles; GPSIMD multiply
  oproj: zero-padded per-pair Wo' (K=128) accumulating in PSUM -> DMA to DRAM
Host: reassemble, add biases, sum height+width partial outputs.
All small DMAs issue from the Sync sequencer so ScalarE stays free for EXP.
"""

import numpy as np
import ml_dtypes

B, H, W, C = 2, 128, 256, 256
HEADS, D = 8, 32
SCALE = float(D) ** -0.5
WC = W // 4   # 64 w-columns per core (height phase)
HC = H // 4   # 32 h-rows per core (width phase)
NTOK = 8192   # tokens per core per phase
BF16 = ml_dtypes.bfloat16

_compiled = {}


def _build_module():
    import contextlib
    import concourse.bass as bass  # noqa: F401
    from concourse import bacc, mybir
    from concourse.tile import TileContext

    bf = mybir.dt.bfloat16
    f32 = mybir.dt.float32
    Exp = mybir.ActivationFunctionType.Exp
    mult = mybir.AluOpType.mult

    nc = bacc.Bacc("TRN2", target_bir_lowering=False)

    # ---- DRAM I/O ----
    xh = nc.dram_tensor("xh", [2, 128, NTOK], bf, kind="ExternalInput")
    xw = nc.dram_tensor("xw", [2, 128, NTOK], bf, kind="ExternalInput")
    wts = {}
    for ph in ("h", "w"):
        wts[f"wq_{ph}"] = nc.dram_tensor(f"wq_{ph}", [2, 128, 256], bf, kind="ExternalInput")
        wts[f"wk_{ph}"] = nc.dram_tensor(f"wk_{ph}", [2, 128, 256], bf, kind="ExternalInput")
        wts[f"wv_{ph}"] = nc.dram_tensor(f"wv_{ph}", [2, 128, 256], bf, kind="ExternalInput")
        wts[f"wo_{ph}"] = nc.dram_tensor(f"wo_{ph}", [4, 2, 128, 128], bf, kind="ExternalInput")
    out_h = nc.dram_tensor("out_h", [2, 128, WC * 128], f32, kind="ExternalOutput")
    out_w = nc.dram_tensor("out_w", [2, 128, HC * 256], f32, kind="ExternalOutput")

    def phase(tc, ctx, xT_dram, wq_d, wk_d, wv_d, wo_d, out_d, is_width,
              xT_pre=None, wts_pre=None, preload_cb=None):
        tag = "w" if is_width else "h"
        XBLK = 256 if is_width else 128          # attention span per block
        nblk = HC if is_width else WC            # 32 or 64 blocks
        half = nblk // 2
        AVW = 4 * XBLK                           # av psum width: 4 pairs

        pool = ctx.enter_context(tc.tile_pool(name="persist", bufs=1))
        pb_pool = ctx.enter_context(tc.tile_pool(name="pb", bufs=22 if is_width else 28))
        work = ctx.enter_context(tc.tile_pool(name="work", bufs=3))
        bc_pool = ctx.enter_context(tc.tile_pool(name="bcp", bufs=5))
        rwork = ctx.enter_context(tc.tile_pool(name="rwork", bufs=1 if is_width else 2))
        at_pool = ctx.enter_context(tc.tile_pool(name="at", bufs=3))
        ps_s = ctx.enter_context(tc.tile_pool(name="ps_s", bufs=1, space="PSUM"))
        ps_av = ctx.enter_context(tc.tile_pool(name="ps_av", bufs=2, space="PSUM"))
        ps_o = ctx.enter_context(tc.tile_pool(name="ps_o", bufs=2, space="PSUM"))

        # ---- weights + xT to SBUF ----
        _ldq = [0]

        def load(dram_ap, shape, nm):
            t = pool.tile(shape, bf, tag=nm, name=nm)
            q = nc.sync if _ldq[0] % 2 == 0 else nc.scalar
            _ldq[0] += 1
            q.dma_start(t[:], dram_ap)
            return t

        if wts_pre is not None:
            wq_sb, wk_sb, wv_sb, wo_sb = wts_pre
        else:
            wq_sb = [load(wq_d[cc], [128, 256], f"wq{cc}") for cc in range(2)]
            wk_sb = [load(wk_d[cc], [128, 256], f"wk{cc}") for cc in range(2)]
            wv_sb = [load(wv_d[cc], [128, 256], f"wv{cc}") for cc in range(2)]
            wo_sb = [[load(wo_d[p, cc], [128, 128], f"wo{p}_{cc}") for cc in range(2)]
                     for p in range(4)]
        if xT_pre is not None:
            xT = xT_pre
        else:
            xT = [load(xT_dram[cc], [128, NTOK], f"xT{cc}") for cc in range(2)]
        if preload_cb is not None:
            preload_cb()

        # ---- v_aug token tiles (zeroed, per-head [v_h | 1 | 0...]) ----
        vaug = {}

        def make_vaug(t_i):
            if t_i in vaug:
                return
            vt = pool.tile([128, 264], bf, tag=f"vaug{t_i}", name=f"vaug{t_i}")
            ps = ps_o.tile([128, 256], f32, tag="ps_misc", name="ps_misc")
            for cc in range(2):
                nc.tensor.matmul(
                    ps[:], xT[cc][:, t_i * 128:(t_i + 1) * 128], wv_sb[cc][:],
                    start=(cc == 0), stop=(cc == 1))
            veng = nc.vector if t_i % 2 == 0 else nc.scalar
            if veng is nc.vector:
                veng.tensor_copy(
                    vt[:].rearrange("p (h t) -> p h t", t=33)[:, :, 0:32],
                    ps[:].rearrange("p (h d) -> p h d", d=32))
            else:
                veng.copy(
                    vt[:].rearrange("p (h t) -> p h t", t=33)[:, :, 0:32],
                    ps[:].rearrange("p (h d) -> p h d", d=32))
            nc.vector.memset(vt[:].rearrange("p (h t) -> p h t", t=33)[:, :, 32], 1.0)
            vaug[t_i] = vt

        # ---- attention main loops ----
        dn = pool.tile([128, AVW], bf, tag="dn", name="dn")        # compacted denominators (bf16)
        rec = pool.tile([128, AVW], bf, tag="rec", name="rec")      # their reciprocals
        rec_d = nc.dram_tensor(f"rec_dram_{tag}", [128, AVW], bf)   # DRAM bounce for partition-bcast
        if not is_width:
            # stride-4 row tile for the last 16 height blocks -> 8-block
            # tail chunks (32-aligned bases 0/32) to shrink the phase tail
            dn2 = pool.tile([64, AVW], bf, tag="dn2", name="dn2")
            rec2 = pool.tile([64, AVW], bf, tag="rec2", name="rec2")
            rec2_d = nc.dram_tensor(f"rec2_dram_{tag}", [64, AVW], bf)
        pairbufs = {}

        # reciprocal chunks must start on 32-aligned dn partitions, so the
        # width phase spreads its denominator rows at stride 4 (rows 4b,4b+1;
        # 4b+2/3 garbage) to allow 8-block chunks; height uses stride 2.
        CH = 8 if is_width else 16           # blocks per reciprocal chunk
        ROWSTR = 4 if is_width else 2        # dn/rec row stride per block
        GRP = 4 if not is_width else 2       # blocks per projection group (512 tokens)
        bctiles = {}

        def dn_map(blk):
            if not is_width and blk >= nblk - 16:
                return dn2, rec2, rec2_d, 4 * (blk - (nblk - 16))
            return dn, rec, rec_d, ROWSTR * blk

        def issue_bcast(blk):
            _, _, rdd, r0 = dn_map(blk)
            bc = bc_pool.tile([128, AVW], bf, tag="bc", name="bc")
            dma_q = nc.sync if (is_width or blk % 2 == 0) else nc.scalar
            dma_q.dma_start(
                bc[0:64, :],
                rdd[r0:r0 + 1, :].partition_broadcast(64).rearrange("p o f -> p (o f)"))
            dma_q.dma_start(
                bc[64:128, :],
                rdd[r0 + 1:r0 + 2, :].partition_broadcast(64).rearrange("p o f -> p (o f)"))
            bctiles[blk] = bc

        def mult_oproj(blk, pbn_box):
            """Normalize one block with its bcast reciprocals; oproj on group end."""
            bc = bctiles.pop(blk)
            OG = 4 if not is_width else 2    # blocks per oproj group (512 tokens)
            gi = blk % OG
            if gi == 0:
                pbn_box[0] = work.tile([128, 4 * 512], bf, tag="pairbufN", name="pairbufN")
            pbn = pbn_box[0]
            # pbn layout: [128, (p:4) (gi:OG) (x:XBLK)] so pair p spans 512 tokens
            # normalize multiply split across DVE (pairs 0-1) and GpSimd (2-3)
            pv = pbn[:].rearrange("q (p g x) -> q p g x", p=4, g=OG)
            pbv = pairbufs[blk][:].rearrange("q (p x) -> q p x", p=4)
            bcv = bc[:].rearrange("q (p x) -> q p x", p=4)
            nc.vector.tensor_tensor(
                pv[:, 0:2, gi, :], pbv[:, 0:2, :], bcv[:, 0:2, :], mult)
            nc.gpsimd.tensor_tensor(
                pv[:, 2:4, gi, :], pbv[:, 2:4, :], bcv[:, 2:4, :], mult)
            del pairbufs[blk]
            if gi == OG - 1:
                g0 = (blk // OG) * OG        # first block of group
                for cc in range(2):
                    po = ps_o.tile([128, 512], f32, tag="ps_misc", name="ps_misc")
                    for p in range(4):
                        nc.tensor.matmul(
                            po[:],
                            wo_sb[p][cc][:],
                            pbn[:, p * 512:(p + 1) * 512],
                            start=(p == 0), stop=(p == 3))
                    osb = work.tile([128, 512], f32, tag="osb", name="osb")
                    if is_width:
                        nc.vector.tensor_copy(osb[:], po[:])
                    else:
                        nc.scalar.copy(osb[:], po[:])
                    nc.sync.dma_start(
                        out_d[cc][:, g0 * XBLK:g0 * XBLK + 512], osb[:])

        def emit_recip_chain(c0, c1):
            dnt, rct, rdd, rbase = dn_map(c0)
            stride = 4 if dnt is not dn or is_width else 2
            rsl = slice(rbase, rbase + stride * (c1 - c0))
            dnf = rwork.tile([32, AVW], f32, tag="dnf", name="dnf")
            recf = rwork.tile([32, AVW], f32, tag="recf", name="recf")
            nc.vector.tensor_copy(dnf[0:rsl.stop - rsl.start, :], dnt[rsl, :])
            nc.vector.reciprocal_approx_fast(recf[:], dnf[:])
            nc.vector.tensor_copy(rct[rsl, :], recf[0:rsl.stop - rsl.start, :])
            nc.sync.dma_start(rdd[rsl, :], rct[rsl, :])

        if True:
            from collections import deque
            blks = list(range(nblk))
            qk_groups = {}
            pbn_box = [None]
            bq = deque()          # blocks awaiting bcast issue (recip done)
            mq = deque()          # blocks with bcast in flight, awaiting multiply
            pending_recip = None
            for blk in blks:
                if not is_width:
                    make_vaug(blk)
                else:
                    make_vaug(2 * blk)
                    make_vaug(2 * blk + 1)
                # --- grouped q/k projection: [q|k][ic] over GRP blocks ---
                g = blk // GRP
                if g not in qk_groups:
                    gtiles = []
                    for ti, w_sb in ((0, wq_sb), (1, wk_sb)):
                        gt = work.tile([128, 1024], bf, tag=f"qkg{ti}", name=f"qkg{ti}")
                        for ic in range(2):
                            psqk = ps_o.tile([128, 512], f32, tag="ps_misc", name="ps_misc")
                            for cc in range(2):
                                nc.tensor.matmul(
                                    psqk[:],
                                    w_sb[cc][:, ic * 128:(ic + 1) * 128],
                                    xT[cc][:, g * 512:(g + 1) * 512],
                                    start=(cc == 0), stop=(cc == 1))
                            nc.vector.tensor_copy(gt[:, ic * 512:(ic + 1) * 512], psqk[:])
                        gtiles.append(gt)
                    qk_groups = {g: gtiles}      # keep only current group
                qg, kg = qk_groups[g]
                boff = (blk % GRP) * 128 if not is_width else (blk % GRP) * 256
                if not is_width:
                    aT = at_pool.tile([128, 8 * 128], bf, tag="aT", name="aT")
                    ps = ps_s.tile([128, 2048], f32, tag="s_ps", name="s_ps")
                    for h in range(8):
                        th, hh = divmod(h, 4)
                        col = 512 * hh + 128 * th        # bank = row-group
                        nc.tensor.matmul(
                            ps[:, col:col + 128],
                            kg[hh * 32:(hh + 1) * 32, th * 512 + boff: th * 512 + boff + 128],
                            qg[hh * 32:(hh + 1) * 32, th * 512 + boff: th * 512 + boff + 128],
                            start=True, stop=True,
                            tile_position=(hh * 32, 0))
                    # aT col for head h=4*th+hh is 128*h = 512*th + 128*hh
                    nc.scalar.activation(
                        aT[:].rearrange("p (th hh x) -> p hh th x", th=2, hh=4),
                        ps[:].rearrange("p (hh b) -> p hh b", hh=4)[:, :, 0:256]
                             .rearrange("p hh (th x) -> p hh th x", th=2),
                        Exp, scale=SCALE)
                else:
                    aT = at_pool.tile([128, 2 * 8 * 256], bf, tag="aT", name="aT")
                    for yc in range(2):
                        ps = ps_s.tile([128, 2048], f32, tag="s_ps", name="s_ps")
                        for h in range(8):
                            th, hh = divmod(h, 4)
                            col = 512 * hh + 256 * th    # bank = row-group
                            nc.tensor.matmul(
                                ps[:, col:col + 256],
                                kg[hh * 32:(hh + 1) * 32, th * 512 + boff + yc * 128: th * 512 + boff + (yc + 1) * 128],
                                qg[hh * 32:(hh + 1) * 32, th * 512 + boff: th * 512 + boff + 256],
                                start=True, stop=True,
                                tile_position=(hh * 32, 0))
                        nc.scalar.activation(
                            aT[:, yc * 2048:(yc + 1) * 2048].rearrange(
                                "p (th hh x) -> p hh th x", th=2, hh=4),
                            ps[:].rearrange("p (hh b) -> p hh b", hh=4)[:, :, 0:512]
                                 .rearrange("p hh (th x) -> p hh th x", th=2),
                            Exp, scale=SCALE)

                # --- AV with denominator column, 2-head col packing per pair ---
                # (psum tiles are [128, 512] = 1 bank; width splits q in half)
                pb = pb_pool.tile([128, AVW], bf, tag="pairbuf", name="pairbuf")
                for qh in range(2 if is_width else 1):
                    av = ps_av.tile([128, 512], f32, tag="av_ps", name="av_ps")
                    if blk == 0 or (not is_width and blk == 1 and qh == 0):
                        # once per psum slot: zero the junk rows (33..63,
                        # 97..127) so downstream casts never see NaNs
                        nc.vector.memset(av[32:64, :], 0.0)
                        nc.vector.memset(av[96:128, :], 0.0)
                    for p in range(4):
                        for s in range(2):
                            h = 2 * p + s
                            op = 64 * s
                            if not is_width:
                                nc.tensor.matmul(
                                    av[op:op + 33, p * 128:(p + 1) * 128],
                                    vaug[blk][:, h * 33:(h + 1) * 33],
                                    aT[:, h * 128:(h + 1) * 128],
                                    start=True, stop=True,
                                    tile_position=(0, op))
                            else:
                                for yc in range(2):
                                    nc.tensor.matmul(
                                        av[op:op + 33, p * 128:(p + 1) * 128],
                                        vaug[2 * blk + yc][:, h * 33:(h + 1) * 33],
                                        aT[:, yc * 2048 + h * 256 + qh * 128:
                                           yc * 2048 + h * 256 + (qh + 1) * 128],
                                        start=(yc == 0), stop=(yc == 1),
                                        tile_position=(0, op))
                    # stash unnormalized pairs; pb columns stay (p, qh, x) order
                    if not is_width:
                        nc.vector.tensor_copy(pb[:], av[:])
                    else:
                        nc.vector.tensor_copy(
                            pb[:].rearrange("q (p j x) -> q p j x", p=4, j=2)[:, :, qh, :],
                            av[:].rearrange("q (p x) -> q p x", p=4))

                # --- compact denominator rows (partitions 32 & 96) ---
                dnt, _, _, r0 = dn_map(blk)
                cq = nc.scalar if (not is_width and blk % 2 == 0) else nc.sync
                cq.dma_start(
                    dnt[r0:r0 + 2, :],
                    pb[:].rearrange("(a p) f -> a p f", a=2)[:, 32, :])
                pairbufs[blk] = pb

                # --- software-pipelined drain of the previous chunk ---
                if pending_recip is not None:
                    emit_recip_chain(*pending_recip)
                    bq.extend(range(pending_recip[0], pending_recip[1]))
                    pending_recip = None
                if bq:
                    b = bq.popleft()
                    issue_bcast(b)
                    mq.append(b)
                if len(mq) >= 3:
                    mult_oproj(mq.popleft(), pbn_box)
                if not is_width and blk >= nblk - 16:
                    if (blk + 1 - (nblk - 16)) % 8 == 0:
                        pending_recip = (blk + 1 - 8, blk + 1)
                elif (blk + 1) % CH == 0:
                    pending_recip = (blk + 1 - CH, blk + 1)

            # --- tail drain ---
            if pending_recip is not None:
                emit_recip_chain(*pending_recip)
                bq.extend(range(pending_recip[0], pending_recip[1]))
            while bq:
                b = bq.popleft()
                issue_bcast(b)
                mq.append(b)
            while mq:
                mult_oproj(mq.popleft(), pbn_box)

    with TileContext(nc) as tc:
        with contextlib.ExitStack() as c0:
            # width-phase xT lives in an always-live pool so its DMA loads
            # prefetch during the height phase instead of gating the
            # phase transition
            xw_pool = c0.enter_context(tc.tile_pool(name="xw_pre", bufs=1))
            _pends = []

            def pre(dram_ap, shape, nm):
                t = xw_pool.tile(shape, bf, tag=nm, name=nm)
                _pends.append((t, dram_ap))
                return t

            xw_sb = [pre(xw[cc], [128, NTOK], f"xw{cc}") for cc in range(2)]
            wts_w = (
                [pre(wts["wq_w"][cc], [128, 256], f"pwq{cc}") for cc in range(2)],
                [pre(wts["wk_w"][cc], [128, 256], f"pwk{cc}") for cc in range(2)],
                [pre(wts["wv_w"][cc], [128, 256], f"pwv{cc}") for cc in range(2)],
                [[pre(wts["wo_w"][p, cc], [128, 128], f"pwo{p}_{cc}")
                  for cc in range(2)] for p in range(4)],
            )

            def emit_preload():
                for i, (t, dram_ap) in enumerate(_pends):
                    q = nc.sync if i % 2 == 0 else nc.scalar
                    q.dma_start(t[:], dram_ap)
            with contextlib.ExitStack() as c1:
                phase(tc, c1, xh, wts["wq_h"], wts["wk_h"], wts["wv_h"],
                      wts["wo_h"], out_h, is_width=False,
                      preload_cb=emit_preload)
            with contextlib.ExitStack() as c2:
                phase(tc, c2, xw, wts["wq_w"], wts["wk_w"], wts["wv_w"],
                      wts["wo_w"], out_w, is_width=True, xT_pre=xw_sb,
                      wts_pre=wts_w)

    nc.compile()
    return nc


def _prep_weights(inp):
    """Host-side weight layouts, bf16."""
    def chunks(Wm):                      # [256, 256] -> [2, 128, 256] (lhsT chunks)
        return np.ascontiguousarray(Wm.reshape(2, 128, 256)).astype(BF16)

    def wo_aug(Wo):                      # -> [4 pairs, 2 cc, 128 K(padded), 128 M]
        out = np.zeros((4, 2, 128, 128), np.float32)
        for p in range(4):
            for cc in range(2):
                out[p, cc, 0:32, :] = Wo[64 * p:64 * p + 32, cc * 128:(cc + 1) * 128]
                out[p, cc, 64:96, :] = Wo[64 * p + 32:64 * p + 64, cc * 128:(cc + 1) * 128]
        return out.astype(BF16)

    d = {}
    for ph in ("h", "w"):
        d[f"wq_{ph}"] = chunks(np.asarray(inp[f"Wq_{ph}"], np.float32))
        d[f"wk_{ph}"] = chunks(np.asarray(inp[f"Wk_{ph}"], np.float32))
        d[f"wv_{ph}"] = chunks(np.asarray(inp[f"Wv_{ph}"], np.float32))
        d[f"wo_{ph}"] = wo_aug(np.asarray(inp[f"Wo_{ph}"], np.float32))
    return d


def kernel(x, Wq_h, Wk_h, Wv_h, Wo_h, bo_h, Wq_w, Wk_w, Wv_w, Wo_w, bo_w, h, w,
           _trace=False):
    from concourse.bass_utils import run_bass_kernel_spmd

    x = np.asarray(x, np.float32)
    xs = x.reshape(B, H, W, C)
    wd = _prep_weights(dict(Wq_h=Wq_h, Wk_h=Wk_h, Wv_h=Wv_h, Wo_h=Wo_h,
                            Wq_w=Wq_w, Wk_w=Wk_w, Wv_w=Wv_w, Wo_w=Wo_w))

    in_maps = []
    for core in range(8):
        b, j = divmod(core, 4)
        xh_a = xs[b][:, j * WC:(j + 1) * WC, :].transpose(2, 1, 0)   # [C, Wc, H]
        xw_a = xs[b][j * HC:(j + 1) * HC, :, :].transpose(2, 0, 1)   # [C, Hc, W]
        m = dict(wd)
        m["xh"] = np.ascontiguousarray(xh_a).reshape(2, 128, NTOK).astype(BF16)
        m["xw"] = np.ascontiguousarray(xw_a).reshape(2, 128, NTOK).astype(BF16)
        in_maps.append(m)

    if "nc" not in _compiled:
        _compiled["nc"] = _build_module()
    nc = _compiled["nc"]

    kw = {}
    if _trace:
        kw = dict(trace=True, trace_cores=[0])
    res = run_bass_kernel_spmd(nc, in_maps, core_ids=list(range(8)), **kw)
    _compiled["last_result"] = res

    out = np.zeros((B, H, W, C), np.float32)
    for core in range(8):
        b, j = divmod(core, 4)
        oh = np.asarray(res.results[core]["out_h"])   # [2(cc), 128(ci), WC*128(n)]
        ow = np.asarray(res.results[core]["out_w"])   # [2(cc), 128(ci), HC*256(n)]
        # outT[c, n], c = cc*128 + ci; height n = w*128 + r -> [r, w, c]
        oh_t = oh.reshape(256, WC, 128).transpose(2, 1, 0)
        out[b, :, j * WC:(j + 1) * WC, :] += oh_t
        # width n = r*256 + wcol -> [r, wcol, c]
        ow_t = ow.reshape(256, HC, 256).transpose(1, 2, 0)
        out[b, j * HC:(j + 1) * HC, :, :] += ow_t
    out += np.asarray(bo_h, np.float32) + np.asarray(bo_w, np.float32)
    return out.reshape(B, H * W, C)


# revision 31
# speedup vs baseline: 1.1837x; 1.1837x over previous
"""AxialAttention Trainium2 kernel: 8-core SPMD, no collectives.

Sharding: core (b, j) computes height-attention for x[b, :, 64j:64j+64, :]
and width-attention for x[b, 32j:32j+32, :, :]; host sums partial outputs.

Per-phase on-device dataflow (all matmuls bf16, fp32 PSUM):
  xT resident [C=256 (2 part-chunks), tokens=8192]
  qT,kT = W.T @ x   (lhsT = W chunks)            [256, 8192] bf16
  v     = x @ Wv    (lhsT = xT token-tiles), N=256 -> strided copy into
          zeroed v_aug tile [128, 8*64] (per-head [v_h | 1 | 0...])
  scores sT = k-stationary, 4-head row-group packing (K=32)
  aT = exp(scale * sT)  on ScalarE, PSUM->SBUF bf16
  AV: stationary [v_h | 1 | 0] (M=64), 2-head col-group packing -> [ohT_h; denom_h]
  denom rows DMA-compacted (one strided DMA) -> one DVE reciprocal per half-phase
  recip rows DMA partition-broadcast (one DMA) -> bc tiles; GPSIMD multiply
  oproj: zero-padded per-pair Wo' (K=128) accumulating in PSUM -> DMA to DRAM
Host: reassemble, add biases, sum height+width partial outputs.
All small DMAs issue from the Sync sequencer so ScalarE stays free for EXP.
"""

import numpy as np
import ml_dtypes

B, H, W, C = 2, 128, 256, 256
HEADS, D = 8, 32
SCALE = float(D) ** -0.5
WC = W // 4   # 64 w-columns per core (height phase)
HC = H // 4   # 32 h-rows per core (width phase)
NTOK = 8192   # tokens per core per phase
BF16 = ml_dtypes.bfloat16

_compiled = {}


def _build_module():
    import contextlib
    import concourse.bass as bass  # noqa: F401
    from concourse import bacc, mybir
    from concourse.tile import TileContext

    bf = mybir.dt.bfloat16
    f32 = mybir.dt.float32
    Exp = mybir.ActivationFunctionType.Exp
    mult = mybir.AluOpType.mult

    nc = bacc.Bacc("TRN2", target_bir_lowering=False)

    # ---- DRAM I/O ----
    xh = nc.dram_tensor("xh", [2, 128, NTOK], bf, kind="ExternalInput")
    xw = nc.dram_tensor("xw", [2, 128, NTOK], bf, kind="ExternalInput")
    wts = {}
    for ph in ("h", "w"):
        wts[f"wq_{ph}"] = nc.dram_tensor(f"wq_{ph}", [2, 128, 256], bf, kind="ExternalInput")
        wts[f"wk_{ph}"] = nc.dram_tensor(f"wk_{ph}", [2, 128, 256], bf, kind="ExternalInput")
        wts[f"wv_{ph}"] = nc.dram_tensor(f"wv_{ph}", [2, 128, 256], bf, kind="ExternalInput")
        wts[f"wo_{ph}"] = nc.dram_tensor(f"wo_{ph}", [4, 2, 128, 128], bf, kind="ExternalInput")
    out_h = nc.dram_tensor("out_h", [2, 128, WC * 128], f32, kind="ExternalOutput")
    out_w = nc.dram_tensor("out_w", [2, 128, HC * 256], f32, kind="ExternalOutput")

    def phase(tc, ctx, xT_dram, wq_d, wk_d, wv_d, wo_d, out_d, is_width,
              xT_pre=None, wts_pre=None, preload_cb=None):
        tag = "w" if is_width else "h"
        XBLK = 256 if is_width else 128          # attention span per block
        nblk = HC if is_width else WC            # 32 or 64 blocks
        half = nblk // 2
        AVW = 4 * XBLK                           # av psum width: 4 pairs

        pool = ctx.enter_context(tc.tile_pool(name="persist", bufs=1))
        pb_pool = ctx.enter_context(tc.tile_pool(name="pb", bufs=22 if is_width else 28))
        work = ctx.enter_context(tc.tile_pool(name="work", bufs=3))
        bc_pool = ctx.enter_context(tc.tile_pool(name="bcp", bufs=5))
        rwork = ctx.enter_context(tc.tile_pool(name="rwork", bufs=1 if is_width else 2))
        at_pool = ctx.enter_context(tc.tile_pool(name="at", bufs=3))
        ps_s = ctx.enter_context(tc.tile_pool(name="ps_s", bufs=1, space="PSUM"))
        ps_av = ctx.enter_context(tc.tile_pool(name="ps_av", bufs=2, space="PSUM"))
        ps_o = ctx.enter_context(tc.tile_pool(name="ps_o", bufs=2, space="PSUM"))

        # ---- weights + xT to SBUF ----
        _ldq = [0]

        def load(dram_ap, shape, nm):
            t = pool.tile(shape, bf, tag=nm, name=nm)
            q = nc.sync if _ldq[0] % 2 == 0 else nc.scalar
            _ldq[0] += 1
            q.dma_start(t[:], dram_ap)
            return t

        if wts_pre is not None:
            wq_sb, wk_sb, wv_sb, wo_sb = wts_pre
        else:
            wq_sb = [load(wq_d[cc], [128, 256], f"wq{cc}") for cc in range(2)]
            wk_sb = [load(wk_d[cc], [128, 256], f"wk{cc}") for cc in range(2)]
            wv_sb = [load(wv_d[cc], [128, 256], f"wv{cc}") for cc in range(2)]
            wo_sb = [[load(wo_d[p, cc], [128, 128], f"wo{p}_{cc}") for cc in range(2)]
                     for p in range(4)]
        if xT_pre is not None:
            xT = xT_pre
        else:
            xT = [load(xT_dram[cc], [128, NTOK], f"xT{cc}") for cc in range(2)]
        if preload_cb is not None:
            preload_cb()

        # ---- v_aug token tiles (zeroed, per-head [v_h | 1 | 0...]) ----
        vaug = {}

        def make_vaug(t_i):
            if t_i in vaug:
                return
            vt = pool.tile([128, 264], bf, tag=f"vaug{t_i}", name=f"vaug{t_i}")
            ps = ps_o.tile([128, 256], f32, tag="ps_misc", name="ps_misc")
            for cc in range(2):
                nc.tensor.matmul(
                    ps[:], xT[cc][:, t_i * 128:(t_i + 1) * 128], wv_sb[cc][:],
                    start=(cc == 0), stop=(cc == 1))
            veng = nc.vector if t_i % 2 == 0 else nc.scalar
            if veng is nc.vector:
                veng.tensor_copy(
                    vt[:].rearrange("p (h t) -> p h t", t=33)[:, :, 0:32],
                    ps[:].rearrange("p (h d) -> p h d", d=32))
            else:
                veng.copy(
                    vt[:].rearrange("p (h t) -> p h t", t=33)[:, :, 0:32],
                    ps[:].rearrange("p (h d) -> p h d", d=32))
            nc.vector.memset(vt[:].rearrange("p (h t) -> p h t", t=33)[:, :, 32], 1.0)
            vaug[t_i] = vt

        # ---- attention main loops ----
        dn = pool.tile([128, AVW], bf, tag="dn", name="dn")        # compacted denominators (bf16)
        rec = pool.tile([128, AVW], bf, tag="rec", name="rec")      # their reciprocals
        rec_d = nc.dram_tensor(f"rec_dram_{tag}", [128, AVW], bf)   # DRAM bounce for partition-bcast
        if not is_width:
            # stride-4 row tile for the last 16 height blocks -> 8-block
            # tail chunks (32-aligned bases 0/32) to shrink the phase tail
            dn2 = pool.tile([64, AVW], bf, tag="dn2", name="dn2")
            rec2 = pool.tile([64, AVW], bf, tag="rec2", name="rec2")
            rec2_d = nc.dram_tensor(f"rec2_dram_{tag}", [64, AVW], bf)
        pairbufs = {}

        # reciprocal chunks must start on 32-aligned dn partitions, so the
        # width phase spreads its denominator rows at stride 4 (rows 4b,4b+1;
        # 4b+2/3 garbage) to allow 8-block chunks; height uses stride 2.
        CH = 8 if is_width else 16           # blocks per reciprocal chunk
        ROWSTR = 4 if is_width else 2        # dn/rec row stride per block
        GRP = 4 if not is_width else 2       # blocks per projection group (512 tokens)
        bctiles = {}

        def dn_map(blk):
            if not is_width and blk >= nblk - 16:
                return dn2, rec2, rec2_d, 4 * (blk - (nblk - 16))
            return dn, rec, rec_d, ROWSTR * blk

        def issue_bcast(blk):
            _, _, rdd, r0 = dn_map(blk)
            bc = bc_pool.tile([128, AVW], bf, tag="bc", name="bc")
            dma_q = nc.sync if blk % 2 == 0 else nc.scalar
            dma_q.dma_start(
                bc[0:64, :],
                rdd[r0:r0 + 1, :].partition_broadcast(64).rearrange("p o f -> p (o f)"))
            dma_q.dma_start(
                bc[64:128, :],
                rdd[r0 + 1:r0 + 2, :].partition_broadcast(64).rearrange("p o f -> p (o f)"))
            bctiles[blk] = bc

        def mult_oproj(blk, pbn_box):
            """Normalize one block with its bcast reciprocals; oproj on group end."""
            bc = bctiles.pop(blk)
            OG = 4 if not is_width else 2    # blocks per oproj group (512 tokens)
            gi = blk % OG
            if gi == 0:
                pbn_box[0] = work.tile([128, 4 * 512], bf, tag="pairbufN", name="pairbufN")
            pbn = pbn_box[0]
            # pbn layout: [128, (p:4) (gi:OG) (x:XBLK)] so pair p spans 512 tokens
            # normalize multiply split across DVE (pairs 0-1) and GpSimd (2-3)
            pv = pbn[:].rearrange("q (p g x) -> q p g x", p=4, g=OG)
            pbv = pairbufs[blk][:].rearrange("q (p x) -> q p x", p=4)
            bcv = bc[:].rearrange("q (p x) -> q p x", p=4)
            nc.vector.tensor_tensor(
                pv[:, 0:2, gi, :], pbv[:, 0:2, :], bcv[:, 0:2, :], mult)
            nc.gpsimd.tensor_tensor(
                pv[:, 2:4, gi, :], pbv[:, 2:4, :], bcv[:, 2:4, :], mult)
            del pairbufs[blk]
            if gi == OG - 1:
                g0 = (blk // OG) * OG        # first block of group
                for cc in range(2):
                    po = ps_o.tile([128, 512], f32, tag="ps_misc", name="ps_misc")
                    for p in range(4):
                        nc.tensor.matmul(
                            po[:],
                            wo_sb[p][cc][:],
                            pbn[:, p * 512:(p + 1) * 512],
                            start=(p == 0), stop=(p == 3))
                    osb = work.tile([128, 512], f32, tag="osb", name="osb")
                    nc.scalar.copy(osb[:], po[:])
                    nc.sync.dma_start(
                        out_d[cc][:, g0 * XBLK:g0 * XBLK + 512], osb[:])

        def emit_recip_chain(c0, c1):
            dnt, rct, rdd, rbase = dn_map(c0)
            stride = 4 if dnt is not dn or is_width else 2
            rsl = slice(rbase, rbase + stride * (c1 - c0))
            dnf = rwork.tile([32, AVW], f32, tag="dnf", name="dnf")
            recf = rwork.tile([32, AVW], f32, tag="recf", name="recf")
            nc.vector.tensor_copy(dnf[0:rsl.stop - rsl.start, :], dnt[rsl, :])
            nc.vector.reciprocal_approx_fast(recf[:], dnf[:])
            nc.vector.tensor_copy(rct[rsl, :], recf[0:rsl.stop - rsl.start, :])
            nc.sync.dma_start(rdd[rsl, :], rct[rsl, :])

        if True:
            from collections import deque
            blks = list(range(nblk))
            qk_groups = {}
            pbn_box = [None]
            bq = deque()          # blocks awaiting bcast issue (recip done)
            mq = deque()          # blocks with bcast in flight, awaiting multiply
            pending_recip = None
            for blk in blks:
                if not is_width:
                    make_vaug(blk)
                else:
                    make_vaug(2 * blk)
                    make_vaug(2 * blk + 1)
                # --- grouped q/k projection: [q|k][ic] over GRP blocks ---
                g = blk // GRP
                if g not in qk_groups:
                    gtiles = []
                    for ti, w_sb in ((0, wq_sb), (1, wk_sb)):
                        gt = work.tile([128, 1024], bf, tag=f"qkg{ti}", name=f"qkg{ti}")
                        for ic in range(2):
                            psqk = ps_o.tile([128, 512], f32, tag="ps_misc", name="ps_misc")
                            for cc in range(2):
                                nc.tensor.matmul(
                                    psqk[:],
                                    w_sb[cc][:, ic * 128:(ic + 1) * 128],
                                    xT[cc][:, g * 512:(g + 1) * 512],
                                    start=(cc == 0), stop=(cc == 1))
                            nc.vector.tensor_copy(gt[:, ic * 512:(ic + 1) * 512], psqk[:])
                        gtiles.append(gt)
                    qk_groups = {g: gtiles}      # keep only current group
                qg, kg = qk_groups[g]
                boff = (blk % GRP) * 128 if not is_width else (blk % GRP) * 256
                if not is_width:
                    aT = at_pool.tile([128, 8 * 128], bf, tag="aT", name="aT")
                    ps = ps_s.tile([128, 2048], f32, tag="s_ps", name="s_ps")
                    for h in range(8):
                        th, hh = divmod(h, 4)
                        col = 512 * hh + 128 * th        # bank = row-group
                        nc.tensor.matmul(
                            ps[:, col:col + 128],
                            kg[hh * 32:(hh + 1) * 32, th * 512 + boff: th * 512 + boff + 128],
                            qg[hh * 32:(hh + 1) * 32, th * 512 + boff: th * 512 + boff + 128],
                            start=True, stop=True,
                            tile_position=(hh * 32, 0))
                    # aT col for head h=4*th+hh is 128*h = 512*th + 128*hh
                    nc.scalar.activation(
                        aT[:].rearrange("p (th hh x) -> p hh th x", th=2, hh=4),
                        ps[:].rearrange("p (hh b) -> p hh b", hh=4)[:, :, 0:256]
                             .rearrange("p hh (th x) -> p hh th x", th=2),
                        Exp, scale=SCALE)
                else:
                    aT = at_pool.tile([128, 2 * 8 * 256], bf, tag="aT", name="aT")
                    for yc in range(2):
                        ps = ps_s.tile([128, 2048], f32, tag="s_ps", name="s_ps")
                        for h in range(8):
                            th, hh = divmod(h, 4)
                            col = 512 * hh + 256 * th    # bank = row-group
                            nc.tensor.matmul(
                                ps[:, col:col + 256],
                                kg[hh * 32:(hh + 1) * 32, th * 512 + boff + yc * 128: th * 512 + boff + (yc + 1) * 128],
                                qg[hh * 32:(hh + 1) * 32, th * 512 + boff: th * 512 + boff + 256],
                                start=True, stop=True,
                                tile_position=(hh * 32, 0))
                        nc.scalar.activation(
                            aT[:, yc * 2048:(yc + 1) * 2048].rearrange(
                                "p (th hh x) -> p hh th x", th=2, hh=4),
                            ps[:].rearrange("p (hh b) -> p hh b", hh=4)[:, :, 0:512]
                                 .rearrange("p hh (th x) -> p hh th x", th=2),
                            Exp, scale=SCALE)

                # --- AV with denominator column, 2-head col packing per pair ---
                # (psum tiles are [128, 512] = 1 bank; width splits q in half)
                pb = pb_pool.tile([128, AVW], bf, tag="pairbuf", name="pairbuf")
                for qh in range(2 if is_width else 1):
                    av = ps_av.tile([128, 512], f32, tag="av_ps", name="av_ps")
                    if blk == 0 or (not is_width and blk == 1 and qh == 0):
                        # once per psum slot: zero the junk rows (33..63,
                        # 97..127) so downstream casts never see NaNs
                        nc.vector.memset(av[32:64, :], 0.0)
                        nc.vector.memset(av[96:128, :], 0.0)
                    for p in range(4):
                        for s in range(2):
                            h = 2 * p + s
                            op = 64 * s
                            if not is_width:
                                nc.tensor.matmul(
                                    av[op:op + 33, p * 128:(p + 1) * 128],
                                    vaug[blk][:, h * 33:(h + 1) * 33],
                                    aT[:, h * 128:(h + 1) * 128],
                                    start=True, stop=True,
                                    tile_position=(0, op))
                            else:
                                for yc in range(2):
                                    nc.tensor.matmul(
                                        av[op:op + 33, p * 128:(p + 1) * 128],
                                        vaug[2 * blk + yc][:, h * 33:(h + 1) * 33],
                                        aT[:, yc * 2048 + h * 256 + qh * 128:
                                           yc * 2048 + h * 256 + (qh + 1) * 128],
                                        start=(yc == 0), stop=(yc == 1),
                                        tile_position=(0, op))
                    # stash unnormalized pairs; pb columns stay (p, qh, x) order
                    if not is_width:
                        nc.vector.tensor_copy(pb[:], av[:])
                    else:
                        nc.vector.tensor_copy(
                            pb[:].rearrange("q (p j x) -> q p j x", p=4, j=2)[:, :, qh, :],
                            av[:].rearrange("q (p x) -> q p x", p=4))

                # --- compact denominator rows (partitions 32 & 96) ---
                dnt, _, _, r0 = dn_map(blk)
                cq = nc.sync
                cq.dma_start(
                    dnt[r0:r0 + 2, :],
                    pb[:].rearrange("(a p) f -> a p f", a=2)[:, 32, :])
                pairbufs[blk] = pb

                # --- software-pipelined drain of the previous chunk ---
                if pending_recip is not None:
                    emit_recip_chain(*pending_recip)
                    bq.extend(range(pending_recip[0], pending_recip[1]))
                    pending_recip = None
                if bq:
                    b = bq.popleft()
                    issue_bcast(b)
                    mq.append(b)
                if len(mq) >= 3:
                    mult_oproj(mq.popleft(), pbn_box)
                if not is_width and blk >= nblk - 16:
                    if (blk + 1 - (nblk - 16)) % 8 == 0:
                        pending_recip = (blk + 1 - 8, blk + 1)
                elif (blk + 1) % CH == 0:
                    pending_recip = (blk + 1 - CH, blk + 1)

            # --- tail drain ---
            if pending_recip is not None:
                emit_recip_chain(*pending_recip)
                bq.extend(range(pending_recip[0], pending_recip[1]))
            while bq:
                b = bq.popleft()
                issue_bcast(b)
                mq.append(b)
            while mq:
                mult_oproj(mq.popleft(), pbn_box)

    with TileContext(nc) as tc:
        with contextlib.ExitStack() as c0:
            # width-phase xT lives in an always-live pool so its DMA loads
            # prefetch during the height phase instead of gating the
            # phase transition
            xw_pool = c0.enter_context(tc.tile_pool(name="xw_pre", bufs=1))
            _pends = []

            def pre(dram_ap, shape, nm):
                t = xw_pool.tile(shape, bf, tag=nm, name=nm)
                _pends.append((t, dram_ap))
                return t

            xw_sb = [pre(xw[cc], [128, NTOK], f"xw{cc}") for cc in range(2)]
            wts_w = (
                [pre(wts["wq_w"][cc], [128, 256], f"pwq{cc}") for cc in range(2)],
                [pre(wts["wk_w"][cc], [128, 256], f"pwk{cc}") for cc in range(2)],
                [pre(wts["wv_w"][cc], [128, 256], f"pwv{cc}") for cc in range(2)],
                [[pre(wts["wo_w"][p, cc], [128, 128], f"pwo{p}_{cc}")
                  for cc in range(2)] for p in range(4)],
            )

            def emit_preload():
                for i, (t, dram_ap) in enumerate(_pends):
                    q = nc.sync if i % 2 == 0 else nc.scalar
                    q.dma_start(t[:], dram_ap)
            with contextlib.ExitStack() as c1:
                phase(tc, c1, xh, wts["wq_h"], wts["wk_h"], wts["wv_h"],
                      wts["wo_h"], out_h, is_width=False,
                      preload_cb=emit_preload)
            with contextlib.ExitStack() as c2:
                phase(tc, c2, xw, wts["wq_w"], wts["wk_w"], wts["wv_w"],
                      wts["wo_w"], out_w, is_width=True, xT_pre=xw_sb,
                      wts_pre=wts_w)

    nc.compile()
    return nc


def _prep_weights(inp):
    """Host-side weight layouts, bf16."""
    def chunks(Wm):                      # [256, 256] -> [2, 128, 256] (lhsT chunks)
        return np.ascontiguousarray(Wm.reshape(2, 128, 256)).astype(BF16)

    def wo_aug(Wo):                      # -> [4 pairs, 2 cc, 128 K(padded), 128 M]
        out = np.zeros((4, 2, 128, 128), np.float32)
        for p in range(4):
            for cc in range(2):
                out[p, cc, 0:32, :] = Wo[64 * p:64 * p + 32, cc * 128:(cc + 1) * 128]
                out[p, cc, 64:96, :] = Wo[64 * p + 32:64 * p + 64, cc * 128:(cc + 1) * 128]
        return out.astype(BF16)

    d = {}
    for ph in ("h", "w"):
        d[f"wq_{ph}"] = chunks(np.asarray(inp[f"Wq_{ph}"], np.float32))
        d[f"wk_{ph}"] = chunks(np.asarray(inp[f"Wk_{ph}"], np.float32))
        d[f"wv_{ph}"] = chunks(np.asarray(inp[f"Wv_{ph}"], np.float32))
        d[f"wo_{ph}"] = wo_aug(np.asarray(inp[f"Wo_{ph}"], np.float32))
    return d


def kernel(x, Wq_h, Wk_h, Wv_h, Wo_h, bo_h, Wq_w, Wk_w, Wv_w, Wo_w, bo_w, h, w,
           _trace=False):
    from concourse.bass_utils import run_bass_kernel_spmd

    x = np.asarray(x, np.float32)
    xs = x.reshape(B, H, W, C)
    wd = _prep_weights(dict(Wq_h=Wq_h, Wk_h=Wk_h, Wv_h=Wv_h, Wo_h=Wo_h,
                            Wq_w=Wq_w, Wk_w=Wk_w, Wv_w=Wv_w, Wo_w=Wo_w))

    in_maps = []
    for core in range(8):
        b, j = divmod(core, 4)
        xh_a = xs[b][:, j * WC:(j + 1) * WC, :].transpose(2, 1, 0)   # [C, Wc, H]
        xw_a = xs[b][j * HC:(j + 1) * HC, :, :].transpose(2, 0, 1)   # [C, Hc, W]
        m = dict(wd)
        m["xh"] = np.ascontiguousarray(xh_a).reshape(2, 128, NTOK).astype(BF16)
        m["xw"] = np.ascontiguousarray(xw_a).reshape(2, 128, NTOK).astype(BF16)
        in_maps.append(m)

    if "nc" not in _compiled:
        _compiled["nc"] = _build_module()
    nc = _compiled["nc"]

    kw = {}
    if _trace:
        kw = dict(trace=True, trace_cores=[0])
    res = run_bass_kernel_spmd(nc, in_maps, core_ids=list(range(8)), **kw)
    _compiled["last_result"] = res

    out = np.zeros((B, H, W, C), np.float32)
    for core in range(8):
        b, j = divmod(core, 4)
        oh = np.asarray(res.results[core]["out_h"])   # [2(cc), 128(ci), WC*128(n)]
        ow = np.asarray(res.results[core]["out_w"])   # [2(cc), 128(ci), HC*256(n)]
        # outT[c, n], c = cc*128 + ci; height n = w*128 + r -> [r, w, c]
        oh_t = oh.reshape(256, WC, 128).transpose(2, 1, 0)
        out[b, :, j * WC:(j + 1) * WC, :] += oh_t
        # width n = r*256 + wcol -> [r, wcol, c]
        ow_t = ow.reshape(256, HC, 256).transpose(1, 2, 0)
        out[b, j * HC:(j + 1) * HC, :, :] += ow_t
    out += np.asarray(bo_h, np.float32) + np.asarray(bo_w, np.float32)
    return out.reshape(B, H * W, C)
